# revision 5
# baseline (speedup 1.0000x reference)
"""Trainium2 Bass kernel for nn_EnhancementGenerator.

Math: the reference is a (buggy, non-recurrent) bidirectional 2-layer GRU
applied pointwise over (B,T), followed by an efficient-kan KANLinear and
1.2*sigmoid(slope*out).  Everything is row-pointwise except that the
backward direction pairs output row (b,t) with input row (b,T-1-t).

Reformulation (validated to ~1e-6 rel against the jax reference):
  * GRU: no recurrence => 4 independent "cells".  Layer-0 sees h=0.  Both
    directions are packed into [f(40); b(40)] = 80-partition tiles; the
    b-direction consumes the same rows as f and the time reversal is applied
    once at feat-assembly with a reversed free-dim access pattern.
  * h1 is carried negated (h1n = (z1-1)*n1) so it costs one fused
    scalar_tensor_tensor op; the L1 recurrent weights are negated on host.
  * KAN spline branch: uniform-knot B-splines == truncated cubic powers.
    feat = GRU output lies strictly in (-1,1), so of the 12 knots only
    {-0.6,-0.2,0.2,0.6} produce kinks; the rest fold into one cubic
    polynomial with matrix coefficients.  spl = A1@feat + A2@feat^2 +
    A3@feat^3 + sum_j W_j @ relu(feat - t_j)^3 + const-bias.  A*/W_j/bias
    are folded on the host from spline_weight*scaler (and slope).

Layout: features/gates in SBUF partitions, rows in the free dim.  Each core
gets 8 batch samples = 8000 rows, processed as 16 row-tiles of 500.
"""
import os
import sys

for _p in (
    "/root/.axon_site",
    "/root/.axon_site/_ro/trn_rl_repo",
    "/root/.axon_site/_ro/pypackages",
    "/opt/trn_rl_repo",
    "/opt/pypackages",
):
    if os.path.isdir(_p) and _p not in sys.path:
        sys.path.append(_p)

import numpy as np

import concourse.bass as bass
import concourse.tile as tile
from concourse import bacc, mybir
from concourse.bass_utils import run_bass_kernel_spmd

F32 = mybir.dt.float32
AF = mybir.ActivationFunctionType
ALU = mybir.AluOpType

N_CORES = 8
B, T, IN_SIZE, HID, OUT_SIZE = 64, 1000, 257, 40, 257
KPAD = 264          # input features padded to 128+128+8
OPAD = 264          # output features padded to 128+128+8
NT = 500            # rows per tile (half of one sample)
SPB = B // N_CORES  # samples per core
ROWS = SPB * T      # rows per core
KCH = [(0, 128), (128, 128), (256, 8)]   # K chunks of padded input
MCH = [(0, 128), (128, 128), (256, 8)]   # M chunks of padded output
KNOTS = [-0.6, -0.2, 0.2, 0.6]           # interior kink knots (j=4..7)
PG = 104            # packed direction block: f at 0:40, b at 64:104 (base-partition rule)
BO = 64             # b-direction partition offset


# --------------------------------------------------------------------------
# host-side weight folding
# --------------------------------------------------------------------------
def fold_weights(inp):
    from math import comb
    W = {k: np.asarray(v, dtype=np.float64) for k, v in inp.items()}
    out = {}
    # gi weights: (KPAD, 6*PG), col block (l*3+g)*PG: f at +0:40, b at +BO:BO+40
    wgi = np.zeros((KPAD, 6 * PG))
    for l in range(2):
        for g in range(3):
            c0 = (l * 3 + g) * PG
            wgi[:IN_SIZE, c0:c0 + 40] = W["Wih_f"][l][g * 40:(g + 1) * 40].T
            wgi[:IN_SIZE, c0 + BO:c0 + BO + 40] = W["Wih_b"][l][g * 40:(g + 1) * 40].T
    out["wgi"] = wgi
    # gh (negated, blockdiag): (PG, 3*PG)
    wgh = np.zeros((PG, 3 * PG))
    for g in range(3):
        wgh[0:40, g * PG:g * PG + 40] = -W["Whh_f"][1][g * 40:(g + 1) * 40].T
        wgh[BO:BO + 40, g * PG + BO:g * PG + BO + 40] = -W["Whh_b"][1][g * 40:(g + 1) * 40].T
    out["wgh"] = wgh
    # gru biases: (PG, 8)
    bg = np.zeros((PG, 8))
    for l in range(2):
        for gi_ in range(2):
            bg[0:40, l * 4 + gi_] = (W["bih_f"][l][gi_ * 40:(gi_ + 1) * 40]
                                     + W["bhh_f"][l][gi_ * 40:(gi_ + 1) * 40])
            bg[BO:BO + 40, l * 4 + gi_] = (W["bih_b"][l][gi_ * 40:(gi_ + 1) * 40]
                                           + W["bhh_b"][l][gi_ * 40:(gi_ + 1) * 40])
        bg[0:40, l * 4 + 2] = W["bhh_f"][l][80:120]
        bg[BO:BO + 40, l * 4 + 2] = W["bhh_b"][l][80:120]
        bg[0:40, l * 4 + 3] = W["bih_f"][l][80:120]
        bg[BO:BO + 40, l * 4 + 3] = W["bih_b"][l][80:120]
    out["bgru"] = bg
    # KAN: truncated-power reformulation
    h = 0.4
    t = -2.2 + h * np.arange(12)
    w = W["spline_weight"] * W["spline_scaler"][..., None]          # (257, 80, 8)
    s = np.zeros((8, 12))
    for m in range(8):
        for k in range(5):
            s[m, m + k] = ((-1) ** k) * comb(4, k) / (6 * h ** 3)
    V = np.einsum("oim,mj->oij", w, s)                              # (257, 80, 12)
    A = np.zeros((4, 257, 80))
    for j in range(4):
        for d in range(4):
            A[d] += V[:, :, j] * comb(3, d) * ((-t[j]) ** (3 - d))
    slope = W["slope"]
    # wkan: (PG, 8*OPAD): idx blocks [base, A1, A2, A3, W4..W7]; feature rows
    # are laid out like feat tiles: hf at 0:40, hb at BO:BO+40.
    wkan = np.zeros((PG, 8 * OPAD))
    mats = [W["base_weight"].T, A[1].T, A[2].T, A[3].T] + [V[:, :, j].T for j in range(4, 8)]
    for idx, m in enumerate(mats):  # m: (80, 257)
        ms = m * slope[None, :]
        wkan[0:40, idx * OPAD:idx * OPAD + OUT_SIZE] = ms[0:40]
        wkan[BO:BO + 40, idx * OPAD:idx * OPAD + OUT_SIZE] = ms[40:80]
    out["wkan"] = wkan
    bk = np.zeros((128, 3))
    a0 = A[0].sum(axis=1) * slope                                    # (257,)
    bk[0:128, 0] = a0[0:128]
    bk[0:128, 1] = a0[128:256]
    bk[0:1, 2] = a0[256:257]
    out["bkan"] = bk
    return {k: np.ascontiguousarray(v, dtype=np.float32) for k, v in out.items()}


# --------------------------------------------------------------------------
# device kernel
# --------------------------------------------------------------------------
def build_nc(n_samples=SPB, q_on_act=2):
    """Build + compile the per-core Bass program (same on all 8 cores)."""
    rows = n_samples * T
    nc = bacc.Bacc("TRN2", target_bir_lowering=False, debug=False)
    xt_d = nc.dram_tensor("xt", [KPAD, rows], F32, kind="ExternalInput")
    wgi_d = nc.dram_tensor("wgi", [KPAD, 6 * PG], F32, kind="ExternalInput")
    wgh_d = nc.dram_tensor("wgh", [PG, 3 * PG], F32, kind="ExternalInput")
    wkan_d = nc.dram_tensor("wkan", [PG, 8 * OPAD], F32, kind="ExternalInput")
    bgru_d = nc.dram_tensor("bgru", [PG, 8], F32, kind="ExternalInput")
    bkan_d = nc.dram_tensor("bkan", [128, 3], F32, kind="ExternalInput")
    yt_d = nc.dram_tensor("yt", [OPAD, rows], F32, kind="ExternalOutput")

    with tile.TileContext(nc) as tc:
        with (
            tc.tile_pool(name="wts", bufs=1) as wp,
            tc.tile_pool(name="xin", bufs=2) as xp,
            tc.tile_pool(name="work", bufs=2) as kp,
            tc.tile_pool(name="feat", bufs=2) as fp,
            tc.tile_pool(name="outp", bufs=3) as op_,
            tc.tile_pool(name="psg", bufs=1, space="PSUM") as psg,
            tc.tile_pool(name="psk", bufs=3, space="PSUM") as psk,
        ):
            # ---- resident weights
            wgi_sb = []
            for ci, (k0, ksz) in enumerate(KCH):
                wt = wp.tile([ksz, 6 * PG], F32, tag=f"wgi{ci}")
                nc.sync.dma_start(wt[:], wgi_d[k0:k0 + ksz, :])
                wgi_sb.append(wt)
            wgh_sb = wp.tile([PG, 3 * PG], F32, tag="wgh")
            nc.sync.dma_start(wgh_sb[:], wgh_d[:])
            wkan_sb = wp.tile([PG, 8 * OPAD], F32, tag="wkan")
            nc.sync.dma_start(wkan_sb[:], wkan_d[:])
            bg = wp.tile([PG, 8], F32, tag="bgru")
            nc.sync.dma_start(bg[:], bgru_d[:])
            bk = wp.tile([128, 3], F32, tag="bkan")
            nc.sync.dma_start(bk[:], bkan_d[:])

            for smp in range(n_samples):
                s0 = smp * T
                xs = []
                for ci, (k0, ksz) in enumerate(KCH):
                    xtile = xp.tile([ksz, T], F32, tag=f"x{ci}")
                    nc.sync.dma_start(xtile[:], xt_d[k0:k0 + ksz, s0:s0 + T])
                    xs.append(xtile)
                feat = [fp.tile([PG, NT], F32, tag=f"feat{h}", name=f"feat{h}")
                        for h in range(2)]

                # ---------------- GRU for both halves
                for h in range(2):
                    xc = [x[:, h * NT:(h + 1) * NT] for x in xs]

                    def gi_matmuls(p, lyr, g, extra=False):
                        c0 = (lyr * 3 + g) * PG
                        for ci in range(3):
                            nc.tensor.matmul(
                                p[:], wgi_sb[ci][:, c0:c0 + PG], xc[ci],
                                start=(ci == 0), stop=(ci == 2 and not extra))

                    # L0
                    ps_r = psg.tile([PG, NT], F32, tag="psA")
                    gi_matmuls(ps_r, 0, 0)
                    ps_z = psg.tile([PG, NT], F32, tag="psB")
                    gi_matmuls(ps_z, 0, 1)
                    ps_n = psg.tile([PG, NT], F32, tag="psC")
                    gi_matmuls(ps_n, 0, 2)
                    rt = kp.tile([PG, NT], F32, tag="rt")
                    nc.scalar.activation(rt[:], ps_r[:], AF.Sigmoid, bias=bg[:, 0:1])
                    zt = kp.tile([PG, NT], F32, tag="zt")
                    nc.scalar.activation(zt[:], ps_z[:], AF.Sigmoid, bias=bg[:, 1:2])
                    ut = kp.tile([PG, NT], F32, tag="ut")
                    nc.vector.scalar_tensor_tensor(
                        ut[:], rt[:], bg[:, 2:3], ps_n[:], op0=ALU.mult, op1=ALU.add)
                    n1 = kp.tile([PG, NT], F32, tag="n1")
                    nc.scalar.activation(n1[:], ut[:], AF.Tanh, bias=bg[:, 3:4])
                    h1n = kp.tile([PG, NT], F32, tag="h1n")
                    nc.vector.scalar_tensor_tensor(
                        h1n[:], zt[:], 1.0, n1[:], op0=ALU.subtract, op1=ALU.mult)
                    # L1
                    ps_r2 = psg.tile([PG, NT], F32, tag="psA")
                    gi_matmuls(ps_r2, 1, 0, extra=True)
                    nc.tensor.matmul(ps_r2[:], wgh_sb[:, 0:PG], h1n[:], start=False, stop=True)
                    ps_z2 = psg.tile([PG, NT], F32, tag="psB")
                    gi_matmuls(ps_z2, 1, 1, extra=True)
                    nc.tensor.matmul(ps_z2[:], wgh_sb[:, PG:2 * PG], h1n[:], start=False, stop=True)
                    ps_n2 = psg.tile([PG, NT], F32, tag="psC")
                    gi_matmuls(ps_n2, 1, 2)
                    ps_p3 = psg.tile([PG, NT], F32, tag="psD")
                    nc.tensor.matmul(ps_p3[:], wgh_sb[:, 2 * PG:3 * PG], h1n[:], start=True, stop=True)
                    r2 = kp.tile([PG, NT], F32, tag="r2")
                    nc.scalar.activation(r2[:], ps_r2[:], AF.Sigmoid, bias=bg[:, 4:5])
                    z2 = kp.tile([PG, NT], F32, tag="z2")
                    nc.scalar.activation(z2[:], ps_z2[:], AF.Sigmoid, bias=bg[:, 5:6])
                    t2 = kp.tile([PG, NT], F32, tag="t2")
                    nc.vector.scalar_tensor_tensor(
                        t2[:], ps_p3[:], bg[:, 6:7], r2[:], op0=ALU.add, op1=ALU.mult)
                    v = kp.tile([PG, NT], F32, tag="v")
                    nc.vector.tensor_add(v[:], t2[:], ps_n2[:])
                    n2 = kp.tile([PG, NT], F32, tag="n2")
                    nc.scalar.activation(n2[:], v[:], AF.Tanh, bias=bg[:, 7:8])
                    dn = kp.tile([PG, NT], F32, tag="dn")
                    nc.vector.tensor_add(dn[:], h1n[:], n2[:])
                    e = kp.tile([PG, NT], F32, tag="e")
                    nc.vector.tensor_mul(e[:], z2[:], dn[:])
                    # hf = n2 - e ; f-half stays at (smp,h), b-half goes
                    # time-reversed into the OTHER half's feat tile.
                    # cover [0:64] so the pad gap 40:64 is defined (zeros)
                    nc.vector.tensor_sub(feat[h][0:64, :], n2[0:64, :], e[0:64, :])
                    nc.vector.tensor_sub(
                        feat[1 - h][BO:BO + 40, :], n2[BO:BO + 40, ::-1], e[BO:BO + 40, ::-1])

                # ---------------- KAN for both halves
                for h in range(2):
                    f = feat[h]
                    sg = kp.tile([PG, NT], F32, tag="sg")
                    nc.scalar.activation(sg[:], f[:], AF.Sigmoid)
                    sl = kp.tile([PG, NT], F32, tag="sl")
                    nc.gpsimd.tensor_mul(sl[:], sg[:], f[:])
                    s2 = kp.tile([PG, NT], F32, tag="s2")
                    nc.scalar.activation(s2[:], f[:], AF.Square)
                    s3 = kp.tile([PG, NT], F32, tag="s3")
                    nc.vector.tensor_mul(s3[:], s2[:], f[:])
                    rhs_list = [sl, f, s2, s3]
                    for ji, tj in enumerate(KNOTS):
                        rj = kp.tile([PG, NT], F32, tag=f"rj{ji}")
                        nc.vector.tensor_scalar(
                            rj[:], f[:], float(tj), 0.0, op0=ALU.subtract, op1=ALU.max)
                        qj = kp.tile([PG, NT], F32, tag=f"qj{ji}")
                        if ji < q_on_act:
                            nc.scalar.activation(qj[:], rj[:], AF.Square)
                        else:
                            nc.vector.tensor_mul(qj[:], rj[:], rj[:])
                        pj = kp.tile([PG, NT], F32, tag=f"pj{ji}")
                        nc.gpsimd.tensor_mul(pj[:], qj[:], rj[:])
                        rhs_list.append(pj)
                    cols = slice(s0 + h * NT, s0 + h * NT + NT)
                    for mc, (m0, msz) in enumerate(MCH):
                        po = psk.tile([msz, NT], F32, tag="kan")
                        for idx, r in enumerate(rhs_list):
                            nc.tensor.matmul(
                                po[:], wkan_sb[:, idx * OPAD + m0:idx * OPAD + m0 + msz],
                                r[:], start=(idx == 0), stop=(idx == 7))
                        ot = op_.tile([msz, NT], F32, tag=f"ot{mc}")
                        nc.scalar.activation(ot[:], po[:], AF.Sigmoid, bias=bk[0:msz, mc:mc + 1])
                        oo = op_.tile([msz, NT], F32, tag=f"oo{mc}")
                        nc.gpsimd.tensor_scalar(oo[:], ot[:], 1.2, None, op0=ALU.mult)
                        nc.sync.dma_start(yt_d[m0:m0 + msz, cols], oo[:])
    nc.compile()
    return nc


# --------------------------------------------------------------------------
# host entry point
# --------------------------------------------------------------------------
_NC_CACHE = {}


def _get_nc(n_samples=SPB):
    if n_samples not in _NC_CACHE:
        _NC_CACHE[n_samples] = build_nc(n_samples)
    return _NC_CACHE[n_samples]


def make_in_maps(inputs, n_samples=SPB, n_cores=N_CORES):
    x = np.asarray(inputs["x"], dtype=np.float32)
    Wf = fold_weights(inputs)
    in_maps = []
    for c in range(n_cores):
        xc = x[c * n_samples:(c + 1) * n_samples].reshape(n_samples * T, IN_SIZE)
        xt = np.zeros((KPAD, n_samples * T), dtype=np.float32)
        xt[:IN_SIZE] = xc.T
        in_maps.append({"xt": np.ascontiguousarray(xt), **Wf})
    return in_maps


def kernel(**inputs):
    x = np.asarray(inputs["x"], dtype=np.float32)
    assert x.shape == (B, T, IN_SIZE), x.shape
    nc = _get_nc(SPB)
    in_maps = make_in_maps(inputs)
    res = run_bass_kernel_spmd(nc, in_maps, list(range(N_CORES)))
    out = np.empty((B, T, OUT_SIZE), dtype=np.float32)
    for c in range(N_CORES):
        yt = res.results[c]["yt"]  # (OPAD, ROWS)
        out[c * SPB:(c + 1) * SPB] = yt[:OUT_SIZE].T.reshape(SPB, T, OUT_SIZE)
    return out


if __name__ == "__main__":
    rng = np.random.default_rng(0)
    demo = {
        "x": rng.standard_normal((B, T, IN_SIZE), dtype=np.float32),
        "Wih_f": rng.standard_normal((2, 120, 257), dtype=np.float32) * 0.1,
        "Whh_f": rng.standard_normal((2, 120, 40), dtype=np.float32) * 0.1,
        "bih_f": rng.standard_normal((2, 120), dtype=np.float32) * 0.1,
        "bhh_f": rng.standard_normal((2, 120), dtype=np.float32) * 0.1,
        "Wih_b": rng.standard_normal((2, 120, 257), dtype=np.float32) * 0.1,
        "Whh_b": rng.standard_normal((2, 120, 40), dtype=np.float32) * 0.1,
        "bih_b": rng.standard_normal((2, 120), dtype=np.float32) * 0.1,
        "bhh_b": rng.standard_normal((2, 120), dtype=np.float32) * 0.1,
        "base_weight": rng.standard_normal((257, 80), dtype=np.float32) * 0.1,
        "spline_weight": rng.standard_normal((257, 80, 8), dtype=np.float32) * 0.1,
        "spline_scaler": np.ones((257, 80), dtype=np.float32),
        "slope": np.ones((257,), dtype=np.float32),
        "lengths": np.full((64,), 1000, dtype=np.int32),
    }
    out = kernel(**demo)
    print("kernel ran, out:", out.shape, out.dtype, float(out.min()), float(out.max()))
